# revision 14
# baseline (speedup 1.0000x reference)
"""Nabla4d forward-difference kernel for 8 Trainium2 NeuronCores.

Full input x: [T=32, Z=64, Y=128, X=128] f32.
Full output:  [4, 32, 64, 128, 128] f32 (stacked fwd diffs along x, y, z, t).

Sharding: T axis split across 8 cores (4 slabs each) + 1-slab halo for the
t-difference. Core 7's halo is a duplicate of the last slab so gt=0 there.

Per-core SBUF layout: one t-slab [Z=64, Y=128, X=128] is viewed as
[128 partitions, 8192 free] with partition p = 2*z + h (h = y//64) and
free f = (y % 64)*128 + x. The DRAM view merges to a contiguous 2-D AP
[(z h), (y x)] = [128, 8192], so every bulk DMA has outer dim 128 and its
descriptors round-robin across all 16 DMA engines (outer dim 2 pins them
to 2 engines at ~27 GB/s each). In this layout:
  gx: free shift by 1 (within x rows)
  gy: free shift by 128; yy=63 boundary rows via two accumulating PE
      matmuls (+a[p+1, yy=0] for even p, -a[p, yy=63]; odd cols zero)
  gz: partition shift by 2 via PE matmul, cols 126/127 zeroed (z=63)
      (compute engines can't shift partitions; the PE can)
  gt: elementwise sub with next slab tile
"""

import numpy as np

import concourse.bass as bass
import concourse.tile as tile
from concourse import bacc, mybir
from concourse.bass_utils import run_bass_kernel_spmd

N_CORES = 8
T_FULL = 32
T_LOC = T_FULL // N_CORES  # 4
TP1 = T_LOC + 1            # 5 (with halo slab)
Z, Y, X = 64, 128, 128
H = 2                      # y halves
P = 128                    # partitions (2*z + h)
F = (Y // H) * X           # 8192 free elems per slab tile
NCH = 4                    # store chunks per slab
CW = F // NCH              # 2048 chunk width (1 MiB per store)
YC = CW // X               # 16 y-rows per chunk
MMN = 512                  # matmul moving free dim (= one PSUM bank of f32)
NMM = CW // MMN            # matmuls per chunk

FP32 = mybir.dt.float32

TRACE = False  # set True (e.g. from test.py) to collect an NTFF profile
LAST_RESULT = None  # BassKernelResults of the most recent run

_NC_CACHE = None


def _rearr(ap3d):
    """[Z, Y, X] dram view -> contiguous [(z h), (y x)] = [128, 8192]."""
    return ap3d.rearrange("z (h y) x -> (z h) (y x)", h=H)


def _build_nc():
    nc = bacc.Bacc(
        "TRN2",
        target_bir_lowering=False,
        debug=False,
        enable_asserts=True,
        num_devices=N_CORES,
    )
    x_t = nc.dram_tensor("x", [TP1, Z, Y, X], FP32, kind="ExternalInput")
    w_t = nc.dram_tensor("w", [P, 3 * P], FP32, kind="ExternalInput")
    o_t = nc.dram_tensor("out", [4, T_LOC, Z, Y, X], FP32, kind="ExternalOutput")
    xap = x_t.ap()
    oap = o_t.ap()

    with tile.TileContext(nc) as tc:
        with (
            tc.tile_pool(name="slab", bufs=3) as slab_pool,
            tc.tile_pool(name="outs", bufs=2) as out_pool,
            tc.tile_pool(name="const", bufs=1) as const_pool,
            tc.tile_pool(name="psum", bufs=4, space=bass.MemorySpace.PSUM) as psum_pool,
            tc.tile_pool(name="psumb", bufs=2, space=bass.MemorySpace.PSUM) as psumb_pool,
        ):
            w_sb = const_pool.tile([P, 3 * P], FP32, tag="w", name="w")
            nc.sync.dma_start(w_sb[:], w_t.ap())

            def load_slab(t):
                # chunked so chunk-0 compute can start after ~1 MiB, not 4 MiB
                sl = slab_pool.tile([P, F], FP32, tag="slab", name=f"slab{t}")
                v = _rearr(xap[t])
                for k in range(NCH):
                    nc.sync.dma_start(
                        sl[:, k * CW : (k + 1) * CW], v[:, k * CW : (k + 1) * CW]
                    )
                return sl

            a = load_slab(0)
            for t in range(T_LOC):
                b = load_slab(t + 1)
                a3 = a[:].rearrange("p (y x) -> p y x", x=X)  # [128, 64, 128]
                for k in range(NCH):
                    c0 = k * CW
                    y0 = k * YC

                    # --- gx: shift by 1 along x (within rows) ---
                    gx = out_pool.tile([P, CW], FP32, tag="gx", name=f"gx{t}_{k}")
                    gx3 = gx[:].rearrange("p (y x) -> p y x", x=X)
                    nc.vector.tensor_sub(
                        gx3[:, :, 0 : X - 1],
                        a3[:, y0 : y0 + YC, 1:X],
                        a3[:, y0 : y0 + YC, 0 : X - 1],
                    )
                    nc.vector.memset(gx3[:, :, X - 1], 0.0)
                    nc.sync.dma_start(_rearr(oap[0, t])[:, c0 : c0 + CW], gx[:])

                    # --- gy: shift by X in free dim (y+1) ---
                    gy = out_pool.tile([P, CW], FP32, tag="gy", name=f"gy{t}_{k}")
                    if k < NCH - 1:
                        nc.vector.tensor_sub(
                            gy[:, :], a[:, c0 + X : c0 + CW + X], a[:, c0 : c0 + CW]
                        )
                    else:
                        nc.vector.tensor_sub(
                            gy[:, 0 : CW - X],
                            a[:, c0 + X : c0 + CW],
                            a[:, c0 : c0 + CW - X],
                        )
                        # yy=63 rows: even p (y=63) take h=1's yy=0 row,
                        # odd p (y=127) are the global boundary (cols zero)
                        psb = psumb_pool.tile([P, X], FP32, tag="psb", name=f"psb{t}")
                        nc.tensor.matmul(
                            psb[:], w_sb[:, P : 2 * P], a[:, 0:X],
                            start=True, stop=False,
                        )
                        nc.tensor.matmul(
                            psb[:], w_sb[:, 2 * P : 3 * P], a[:, F - X : F],
                            start=False, stop=True,
                        )
                        nc.vector.tensor_copy(gy[:, CW - X : CW], psb[:])
                    nc.sync.dma_start(_rearr(oap[1, t])[:, c0 : c0 + CW], gy[:])

                    # --- gz: z+1 partition shift via PE; zero cols give z=63 ---
                    gz = out_pool.tile([P, CW], FP32, tag="gz", name=f"gz{t}_{k}")
                    for j in range(NMM):
                        ps = psum_pool.tile(
                            [P, MMN], FP32, tag="ps", name=f"ps{t}_{k}_{j}"
                        )
                        nc.tensor.matmul(
                            ps[:],
                            w_sb[:, 0:P],
                            a[:, c0 + j * MMN : c0 + (j + 1) * MMN],
                            start=True,
                            stop=True,
                        )
                        nc.vector.tensor_copy(gz[:, j * MMN : (j + 1) * MMN], ps[:])
                    nc.sync.dma_start(_rearr(oap[2, t])[:, c0 : c0 + CW], gz[:])

                    # --- gt: next slab minus this slab ---
                    gt = out_pool.tile([P, CW], FP32, tag="gt", name=f"gt{t}_{k}")
                    nc.vector.tensor_sub(
                        gt[:, :], b[:, c0 : c0 + CW], a[:, c0 : c0 + CW]
                    )
                    nc.sync.dma_start(_rearr(oap[3, t])[:, c0 : c0 + CW], gt[:])
                a = b

    nc.compile()
    return nc


def _make_w():
    """lhsT weights: out[m,f] = sum_k w[k,m]*rhs[k,f].

    cols 0:128   (rhs = a chunk):      gz = a[m+2] - a[m], zero at z=63
    cols 128:256 (rhs = a[:, 0:X]):    gy boundary +a[m+1] for even m
    cols 256:384 (rhs = a[:, F-X:F]):  gy boundary -a[m] for even m
    """
    w = np.zeros((P, 3 * P), dtype=np.float32)
    for m in range(P - 2):
        w[m + 2, m] = 1.0
        w[m, m] = -1.0
    for m in range(0, P, 2):
        w[m + 1, P + m] = 1.0
        w[m, 2 * P + m] = -1.0
    return w


def _get_nc():
    global _NC_CACHE
    if _NC_CACHE is None:
        _NC_CACHE = _build_nc()
    return _NC_CACHE


def kernel(x: np.ndarray) -> np.ndarray:
    global LAST_RESULT
    x = np.ascontiguousarray(x, dtype=np.float32)
    assert x.shape == (T_FULL, Z, Y, X)

    w = _make_w()
    in_maps = []
    for i in range(N_CORES):
        lo = i * T_LOC
        if i < N_CORES - 1:
            xs = x[lo : lo + TP1]
        else:
            xs = np.concatenate([x[lo:], x[-1:]], axis=0)
        in_maps.append({"x": np.ascontiguousarray(xs), "w": w})

    nc = _get_nc()
    res = run_bass_kernel_spmd(
        nc, in_maps, core_ids=list(range(N_CORES)), trace=TRACE
    )
    LAST_RESULT = res

    full = np.empty((4, T_FULL, Z, Y, X), dtype=np.float32)
    for i in range(N_CORES):
        full[:, i * T_LOC : (i + 1) * T_LOC] = res.results[i]["out"]
    return full


# revision 17
# speedup vs baseline: 1.0478x; 1.0478x over previous
"""Nabla4d: Y-sharded, (z, x-half) partition layout, 8 Trainium2 NeuronCores.

Full input x: [T=32, Z=64, Y=128, X=128] f32.
Full output:  [4, 32, 64, 128, 128] f32 (stacked fwd diffs along x, y, z, t).

Sharding: Y axis split across 8 cores (16 planes each). Halo = ONE y-plane
(1 MiB) from the next core; core 7 duplicates its own last plane so its
boundary rows come out zero. T and Z are fully local.

Per-core SBUF layout: partition p = 2*z + xh (xh = x//64), free within a
slab-group g of 8 t-blocks: f = tl*1024 + y*64 + xl (y local 0..15,
xl = x%64). Device x/out are host-packed [4 groups, 128, 8192]: slab loads
are 32 KiB/row descriptors, stores 8 KiB chunk-slices of 32 KiB rows.

Per t-block (1024 free):
  gx: bulk free shift 1 (xl rows); x=63 seam via tiny PE matmul
      (+a[p+1, xl=0] for even p, odd p rows zero = global x boundary)
  gy: bulk free shift 64; y=15 rows = halo tile - a (pure DVE, same
      partitions, no PE)
  gz: partition shift 2 via PE (rows 126/127 zero = global z boundary)
  gt: free shift 1024 (t=31 memset zero)
"""

import numpy as np

import concourse.bass as bass
import concourse.tile as tile
from concourse import bacc, mybir
from concourse.bass_utils import run_bass_kernel_spmd

N_CORES = 8
T, Z, Y, X = 32, 64, 128, 128
YL = Y // N_CORES          # 16 local y planes
P = 128                    # partitions = 2*z + xh
XL = X // 2                # 64 x elems per partition row
FPT = YL * XL              # 1024 free elems per t-block
NG = 4                     # slab groups
TPG = T // NG              # 8 t-blocks per group
F = TPG * FPT              # 8192 free per slab
CW = 2 * FPT               # 2048 store chunk (one t-pair)
MMN = 512                  # matmul moving free dim (= one PSUM bank of f32)

FP32 = mybir.dt.float32

TRACE = False
LAST_RESULT = None

_NC_CACHE = None


def _build_nc():
    nc = bacc.Bacc(
        "TRN2",
        target_bir_lowering=False,
        debug=False,
        enable_asserts=True,
        num_devices=N_CORES,
    )
    x_t = nc.dram_tensor("x", [NG, P, F], FP32, kind="ExternalInput")
    hl_t = nc.dram_tensor("hl", [P, T * XL], FP32, kind="ExternalInput")
    w1_t = nc.dram_tensor("w1", [P, 3 * P], FP32, kind="ExternalInput")
    o_t = nc.dram_tensor("out", [4, NG, P, F], FP32, kind="ExternalOutput")
    xap = x_t.ap()
    oap = o_t.ap()

    with tile.TileContext(nc) as tc:
        with (
            tc.tile_pool(name="slab", bufs=3) as slab_pool,
            tc.tile_pool(name="outs", bufs=2) as out_pool,
            tc.tile_pool(name="const", bufs=1) as const_pool,
            tc.tile_pool(name="psum", bufs=4, space=bass.MemorySpace.PSUM) as psum_pool,
            tc.tile_pool(name="psumb", bufs=2, space=bass.MemorySpace.PSUM) as psumb_pool,
        ):
            w1_sb = const_pool.tile([P, 3 * P], FP32, tag="w1", name="w1")
            nc.sync.dma_start(w1_sb[:], w1_t.ap())
            hl_sb = const_pool.tile([P, T * XL], FP32, tag="hl", name="hl")
            nc.sync.dma_start(hl_sb[:], hl_t.ap())
            hl3 = hl_sb[:].rearrange("p (t xl) -> p t xl", xl=XL)

            def load_slab(g):
                # chunked 8 KiB descriptors: avoids long 32 KiB engine bursts
                sl = slab_pool.tile([P, F], FP32, tag="slab", name=f"slab{g}")
                for k in range(F // CW):
                    nc.sync.dma_start(
                        sl[:, k * CW : (k + 1) * CW], xap[g][:, k * CW : (k + 1) * CW]
                    )
                return sl

            a = load_slab(0)
            for g in range(NG):
                b = load_slab(g + 1) if g < NG - 1 else None
                for jj in range(TPG // 2):
                    o2 = jj * CW
                    gx = out_pool.tile([P, CW], FP32, tag="gx", name=f"gx{g}_{jj}")
                    gy = out_pool.tile([P, CW], FP32, tag="gy", name=f"gy{g}_{jj}")
                    gz = out_pool.tile([P, CW], FP32, tag="gz", name=f"gz{g}_{jj}")
                    gt = out_pool.tile([P, CW], FP32, tag="gt", name=f"gt{g}_{jj}")

                    for tt in range(2):
                        t = g * TPG + 2 * jj + tt
                        o = o2 + tt * FPT
                        a_t = a[:, o : o + FPT].rearrange(
                            "p (y xl) -> p y xl", xl=XL
                        )
                        gx_t = gx[:, tt * FPT : (tt + 1) * FPT].rearrange(
                            "p (y xl) -> p y xl", xl=XL
                        )
                        gy_t = gy[:, tt * FPT : (tt + 1) * FPT].rearrange(
                            "p (y xl) -> p y xl", xl=XL
                        )

                        # --- gx bulk: shift 1 along xl ---
                        nc.vector.tensor_sub(
                            gx_t[:, :, 0 : XL - 1],
                            a_t[:, :, 1:XL],
                            a_t[:, :, 0 : XL - 1],
                        )
                        # x=63 seam: +a[p+1, xl=0] even p, -a[p, xl=63];
                        # odd p rows zero (global x=127)
                        psb = psumb_pool.tile(
                            [P, YL], FP32, tag="psb", name=f"psb{g}_{jj}_{tt}"
                        )
                        nc.tensor.matmul(
                            psb[:], w1_sb[:, P : 2 * P], a_t[:, :, 0],
                            start=True, stop=False,
                        )
                        nc.tensor.matmul(
                            psb[:], w1_sb[:, 2 * P : 3 * P], a_t[:, :, XL - 1],
                            start=False, stop=True,
                        )
                        nc.vector.tensor_copy(gx_t[:, :, XL - 1], psb[:])

                        # --- gy: bulk shift 64; y=15 rows = halo - a ---
                        nc.vector.tensor_sub(
                            gy_t[:, 0 : YL - 1, :],
                            a_t[:, 1:YL, :],
                            a_t[:, 0 : YL - 1, :],
                        )
                        nc.vector.tensor_sub(
                            gy_t[:, YL - 1, :], hl3[:, t, :], a_t[:, YL - 1, :]
                        )

                        # --- gz: z+1 partition shift via PE; rows 126/127 zero ---
                        for u in range(2):
                            ps = psum_pool.tile(
                                [P, MMN], FP32, tag="ps", name=f"ps{g}_{jj}_{tt}_{u}"
                            )
                            nc.tensor.matmul(
                                ps[:], w1_sb[:, 0:P],
                                a[:, o + u * MMN : o + (u + 1) * MMN],
                                start=True, stop=True,
                            )
                            nc.vector.tensor_copy(
                                gz[:, tt * FPT + u * MMN : tt * FPT + (u + 1) * MMN],
                                ps[:],
                            )

                    # --- gt: free shift by 1024; t=31 is zero ---
                    nc.vector.tensor_sub(
                        gt[:, 0:FPT], a[:, o2 + FPT : o2 + CW], a[:, o2 : o2 + FPT]
                    )
                    last_pair = jj == TPG // 2 - 1
                    if not last_pair:
                        nc.vector.tensor_sub(
                            gt[:, FPT:CW],
                            a[:, o2 + CW : o2 + CW + FPT],
                            a[:, o2 + FPT : o2 + CW],
                        )
                    elif b is not None:
                        nc.vector.tensor_sub(
                            gt[:, FPT:CW], b[:, 0:FPT], a[:, o2 + FPT : o2 + CW]
                        )
                    else:
                        nc.vector.memset(gt[:, FPT:CW], 0.0)

                    nc.sync.dma_start(oap[0, g][:, o2 : o2 + CW], gx[:])
                    nc.sync.dma_start(oap[1, g][:, o2 : o2 + CW], gy[:])
                    nc.sync.dma_start(oap[2, g][:, o2 : o2 + CW], gz[:])
                    nc.sync.dma_start(oap[3, g][:, o2 : o2 + CW], gt[:])
                a = b

    nc.compile()
    return nc


def _make_w1():
    """lhsT weights [128, 384]: out[m,f] = sum_k w[k,m]*rhs[k,f].

    cols 0:128   (rhs = a block):            gz = a[m+2] - a[m], rows 126/127 zero
    cols 128:256 (rhs = a_t[:, :, xl=0]):    gx seam +a[m+1] for even m
    cols 256:384 (rhs = a_t[:, :, xl=63]):   gx seam -a[m] for even m only
    Odd m rows of the seam output are all-zero = global x=127 boundary.
    """
    w = np.zeros((P, 3 * P), dtype=np.float32)
    for m in range(P - 2):
        w[m + 2, m] = 1.0
        w[m, m] = -1.0
    for m in range(0, P, 2):
        w[m + 1, P + m] = 1.0
        w[m, 2 * P + m] = -1.0
    return w


def _pack_x(xc):
    """[T, Z, 16, X] y-shard -> [NG, 128, 8192] device layout."""
    v = xc.reshape(T, Z, YL, 2, XL).transpose(1, 3, 0, 2, 4)  # z, xh, t, y, xl
    return np.ascontiguousarray(v.reshape(P, NG, F).transpose(1, 0, 2))


def _pack_halo(plane):
    """[T, Z, X] y-plane -> [128, T*64] device layout (p=2z+xh, f=t*64+xl)."""
    v = plane.reshape(T, Z, 2, XL).transpose(1, 2, 0, 3)  # z, xh, t, xl
    return np.ascontiguousarray(v.reshape(P, T * XL))


def _unpack_out(o):
    """[4, NG, 128, 8192] device layout -> [4, T, Z, 16, X]."""
    v = o.reshape(4, NG, Z, 2, TPG, YL, XL).transpose(0, 1, 4, 2, 5, 3, 6)
    return v.reshape(4, T, Z, YL, X)


def _get_nc():
    global _NC_CACHE
    if _NC_CACHE is None:
        _NC_CACHE = _build_nc()
    return _NC_CACHE


def kernel(x: np.ndarray) -> np.ndarray:
    global LAST_RESULT
    x = np.ascontiguousarray(x, dtype=np.float32)
    assert x.shape == (T, Z, Y, X)

    w1 = _make_w1()
    in_maps = []
    for i in range(N_CORES):
        y0 = i * YL
        xc = x[:, :, y0 : y0 + YL, :]
        plane = x[:, :, y0 + YL, :] if i < N_CORES - 1 else x[:, :, Y - 1, :]
        in_maps.append({"x": _pack_x(xc), "hl": _pack_halo(plane), "w1": w1})

    nc = _get_nc()
    res = run_bass_kernel_spmd(
        nc, in_maps, core_ids=list(range(N_CORES)), trace=TRACE
    )
    LAST_RESULT = res

    full = np.empty((4, T, Z, Y, X), dtype=np.float32)
    for i in range(N_CORES):
        full[:, :, :, i * YL : (i + 1) * YL, :] = _unpack_out(
            res.results[i]["out"]
        )
    return full
